# revision 33
# baseline (speedup 1.0000x reference)
"""CoordinateLSTM cell on 8 Trainium2 NeuronCores (Bass/Tile, data-parallel).

Computes, for B=32768, I=H=128:
    total = concat([x, h], -1)                # [B, 256]
    s1 = sigmoid(total @ W1.T + b1)
    s2 = sigmoid(total @ W2.T + b2)
    fl = tanh   (total @ Wf.T + bf)
    s3 = sigmoid(total @ W3.T + b3)
    new_c = c * s1 + s2 * fl
    new_h = tanh(new_c) * s3

Sharding: batch dim split 8 ways (4096 rows/core); weights replicated.

Per-core kernel structure (per 2048-row chunk, 2 chunks/core):
  - gpsimd (SWDGE) DMA loads x,h with an inline f32->fp16 cast and c with
    an f32->bf16 cast (casts are free in the DMA datapath)
  - one blocked HWDGE xbar DMA-transpose per input per chunk to get
    features onto partitions (out[c, r, p] = in[p, r*128+c])
  - per 128-row subtile: 3 accumulating fp16 matmuls into one PSUM bank
    [128, 512]: xT.T@Wtx + hT.T@Wth + ones.T@bias (rank-1 bias add);
    weights and bias|ones are host-packed into 2 DMA-able tensors
  - ScalarE sigmoid/tanh directly off PSUM (gates packed [s1|s2|s3|fl]),
    written in bf16 so the VectorE combine runs in the DVE 2x perf mode
  - VectorE elementwise combine in bf16; per-512-row-group SWDGE stores
    upcast bf16 -> f32 on the way to HBM
  - hoisted ACT sigmoid/tanh table load at t=0; x-part matmuls emitted
    before h-parts so PE work starts after the first (x) transpose

Perf notes (HW-measured, loop-differencing, ±8 us run noise):
  - the workload is DMA-bound end to end: the IO alone (loads, transposes,
    stores, no compute) measures ~60-68 us vs ~29 us of pure HBM traffic
    at 358 GB/s; per-DMA-op overhead ~1.5-2 us dominates, so few large
    DMAs beat many small ones
  - PE warmup matmuls were removed: under steady state the PE never idles
    long enough for HAM to re-throttle, and the warmup PSUM tile blocked
    half of PSUM every pass
"""

import sys

if "/opt/trn_rl_repo" not in sys.path:
    sys.path.insert(0, "/opt/trn_rl_repo")

import numpy as np
import ml_dtypes

BF16 = ml_dtypes.bfloat16
MM_DT = np.float16  # matmul operand dtype: fp16 = 10-bit mantissa, 1 cyc/row

B, I, H = 32768, 128, 128
N_CORES = 8
B_CORE = B // N_CORES  # 4096
SUB = 128              # rows per matmul tile (M)
G = 512                # stacked gate width: [s1 | s2 | s3 | fl]
SUBS_PER_GROUP = 4     # subtiles per PSUM group (4 banks)
GROUPS_PER_CHUNK = 4
CHUNK = SUB * SUBS_PER_GROUP * GROUPS_PER_CHUNK  # 2048 rows, load granularity

TRACE = False          # set by test.py to profile
LAST_EXEC_NS = None
BIAS_MM = True         # dev knob: emit the rank-1 bias matmul (timing A/B)
WARMUP = 0             # PE warmup matmuls; 0: loop keeps HAM warm, and the
                       # warmup PSUM tile otherwise blocks half of PSUM per pass
FILL_OPT = True        # dev knob: actwarm + x-before-h matmul ordering
BIAS_FIRST = False     # dev knob: emit dep-free bias matmuls at group head
C_BF16 = True          # load c as bf16 (SWDGE cast) -> DVE 2x elig
SIG_BF16 = True        # ACT writes gates in bf16 -> DVE 2x elig
POST_BF16 = True       # elementwise combine + outputs in bf16, SWDGE cast store
TP_ENGINE = "sync"     # engine for both transposes: "scalar"|"sync"|"split"
STORE_EVERY = 1        # store granularity in groups (0 = whole chunk)
PIPE = True            # software-pipeline emission: prefetch chunk n+1 loads
                       # between chunk n's compute groups (Tile schedules by
                       # emission priority, so emission order ~= schedule)
WARMUP_POS = "pre"     # "pre": warmup MMs before chunk-0 loads; "mid": after
IN_BUFS = 3            # xin/xtp/cin pool depth
SIGP_BUFS = 3          # sigp pool depth
POST_BUFS = 3          # post pool depth
TP_SPLIT = 1           # split x/h loads + transposes into this many pieces
                       # per chunk (shortens the load->transpose->matmul
                       # critical chain during fill)
IO_ONLY = False        # debug: emit only the DMA traffic (loads, transposes,
                       # stores of a constant tile) to measure the HW DMA floor
IO_PARTS = "lts"       # under IO_ONLY: l=loads(+c), t=transposes, s=stores
XH_ENGINE = "gpsimd"   # "gpsimd": fp16 cast loads; "sync": plain f32 loads
SWDGE_QUEUES = 1       # SWDGE queue count (plain dma_start always uses q0)
C_ENGINE = "sync"      # c load ring: "sync" f32 HWDGE measured consistently
                       # ~2-4 us faster than SWDGE bf16-cast: it takes 4.2 MB
                       # of reads off the saturated SWDGE queue (m1 then runs
                       # mixed f32xbf16 on DVE, losing 2x on that one op)
ST_ENGINE = "gpsimd"   # store engine: "gpsimd" (bf16->f32 cast) or "sync"

_cache = {}


def _build(rows, reps=1, loop_n=1):
    """Build + compile the per-core Bass program for `rows` rows.

    reps > 1 unrolls the whole computation that many times; loop_n > 1 wraps
    it in a device-side For_i loop. Both are idempotent (same inputs/outputs)
    and exist so wall-clock differencing can recover the pure kernel
    execution time without NTFF profiling.
    """
    import concourse.bacc as bacc
    import concourse.bass as bass
    import concourse.tile as tile
    import concourse.mybir as mybir
    from contextlib import ExitStack, nullcontext

    dt = mybir.dt
    global MM_DT_BIR
    MM_DT_BIR = dt.float16 if MM_DT == np.float16 else dt.bfloat16
    AF = mybir.ActivationFunctionType
    chunk = SUB * SUBS_PER_GROUP * GROUPS_PER_CHUNK
    assert rows % chunk == 0
    n_chunks = rows // chunk
    spc = chunk // SUB  # subtiles per chunk

    nc = bacc.Bacc(
        "TRN2",
        target_bir_lowering=False,
        debug=False,
        enable_asserts=False,
        num_devices=N_CORES,
        num_swdge_queues=SWDGE_QUEUES,
    )
    x_d = nc.dram_tensor("x", [rows, I], dt.float32, kind="ExternalInput")
    h_d = nc.dram_tensor("h", [rows, H], dt.float32, kind="ExternalInput")
    c_d = nc.dram_tensor("c", [rows, H], dt.float32, kind="ExternalInput")
    wtxh_d = nc.dram_tensor("wtxh", [I, 2 * G], MM_DT_BIR, kind="ExternalInput")
    bo_d = nc.dram_tensor("bo", [1, G + SUB], MM_DT_BIR, kind="ExternalInput")
    nh_d = nc.dram_tensor("new_h", [rows, H], dt.float32, kind="ExternalOutput")
    ncv_d = nc.dram_tensor("new_c", [rows, H], dt.float32, kind="ExternalOutput")

    # DRAM slab views: partition p holds `spc` CONSECUTIVE rows (contiguous
    # 4 KiB per partition -> 1 DMA descriptor per partition instead of 8).
    # Logical subtile r of a chunk is the strided row set {spc*p + r}; the
    # same mapping is applied to x, h, c and the outputs, so the matmul /
    # elementwise / store row-identity stays consistent.
    x_r = x_d[:].rearrange("(n p r) c -> n p r c", r=spc, p=SUB)
    h_r = h_d[:].rearrange("(n p r) c -> n p r c", r=spc, p=SUB)
    c_r = c_d[:].rearrange("(n p r) c -> n p r c", r=spc, p=SUB)
    nh_r = nh_d[:].rearrange("(n p r) c -> n p r c", r=spc, p=SUB)
    ncv_r = ncv_d[:].rearrange("(n p r) c -> n p r c", r=spc, p=SUB)

    with tile.TileContext(nc) as tc, ExitStack() as ctx:
        const = ctx.enter_context(tc.tile_pool(name="const", bufs=1))
        wtxh_sb = const.tile([I, 2 * G], MM_DT_BIR)
        nc.sync.dma_start(wtxh_sb[:], wtxh_d[:])
        bo_sb = const.tile([1, G + SUB], MM_DT_BIR)
        nc.sync.dma_start(bo_sb[:], bo_d[:])
        wtx_sb = wtxh_sb[:, 0:G]
        wth_sb = wtxh_sb[:, G : 2 * G]
        bias_sb = bo_sb[:, 0:G]
        ones_sb = bo_sb[:, G : G + SUB]

        xin = ctx.enter_context(tc.tile_pool(name="xin", bufs=IN_BUFS))
        xtp = ctx.enter_context(tc.tile_pool(name="xtp", bufs=IN_BUFS))
        cin = ctx.enter_context(tc.tile_pool(name="cin", bufs=IN_BUFS))
        psum_bufs = max(2, 8 // SUBS_PER_GROUP)
        psum = ctx.enter_context(
            tc.tile_pool(name="psum", bufs=psum_bufs, space=bass.MemorySpace.PSUM)
        )
        sigp = ctx.enter_context(tc.tile_pool(name="sigp", bufs=SIGP_BUFS))
        post = ctx.enter_context(tc.tile_pool(name="post", bufs=POST_BUFS))

        post_dt = dt.bfloat16 if (POST_BF16 and ST_ENGINE == "gpsimd") else dt.float32
        sig_dt = dt.bfloat16 if SIG_BF16 else dt.float32
        c_dt = dt.bfloat16 if (C_BF16 and C_ENGINE == "gpsimd") else dt.float32
        st_eng = {"gpsimd": nc.gpsimd, "sync": nc.sync, "scalar": nc.scalar}[
            ST_ENGINE
        ]
        c_eng = {"gpsimd": nc.gpsimd, "sync": nc.sync, "scalar": nc.scalar}[
            C_ENGINE
        ]

        # Zero tile for PE warmup matmuls (contents irrelevant).
        wu = const.tile([SUB, G], MM_DT_BIR)
        nc.gpsimd.memset(wu[:], 0.0)

        if IO_ONLY:
            # Constant source tiles for stores; data content is irrelevant.
            io_ncw = const.tile([SUB, SUB * SUBS_PER_GROUP * GROUPS_PER_CHUNK // SUB, H], post_dt, name="io_ncw")
            io_nhw = const.tile([SUB, SUB * SUBS_PER_GROUP * GROUPS_PER_CHUNK // SUB, H], post_dt, name="io_nhw")
            nc.gpsimd.memset(io_ncw[:], 0.25)
            nc.gpsimd.memset(io_nhw[:], 0.25)

        # Dummy activation at t=0: walrus inserts the sigmoid/tanh ACT table
        # load right before the first Activation on the ScalarE stream, so
        # this hoists the ~2.6 us table load into the DMA fill phase instead
        # of the first real sigmoid's critical path.
        if FILL_OPT:
            actwarm = const.tile([1, 1], dt.float32)
            nc.scalar.activation(actwarm[:], wu[0:1, 0:1], AF.Sigmoid)

        loop_cm = (
            tc.For_i(0, loop_n, 1, staggered_reset=True)
            if loop_n > 1
            else nullcontext()
        )
        tp_x_eng = nc.scalar if TP_ENGINE in ("scalar", "split") else nc.sync
        tp_h_eng = nc.scalar if TP_ENGINE == "scalar" else nc.sync

        n_pieces = TP_SPLIT if TP_SPLIT else 1
        assert spc % n_pieces == 0
        ppc = spc // n_pieces  # subtiles per piece

        def emit_xh_load(n):
            if IO_ONLY and "l" not in IO_PARTS:
                return None, None
            if XH_ENGINE == "sync":
                # Plain f32 HWDGE loads (measured ~1.5x faster than SWDGE
                # cast loads), then on-chip f32->fp16 cast on the otherwise
                # idle gpsimd engine.
                xf = xin.tile([SUB, spc, I], dt.float32, tag="xbf32", name="xbf32")
                hf = xin.tile([SUB, spc, H], dt.float32, tag="hbf32", name="hbf32")
                nc.sync.dma_start(xf[:], x_r[n])
                nc.sync.dma_start(hf[:], h_r[n])
                if IO_ONLY:
                    return None, None
                xbf = xin.tile([SUB, spc, I], MM_DT_BIR, tag="xbf", name="xbf")
                hbf = xin.tile([SUB, spc, H], MM_DT_BIR, tag="hbf", name="hbf")
                nc.gpsimd.tensor_copy(xbf[:], xf[:])
                nc.gpsimd.tensor_copy(hbf[:], hf[:])
                return xbf, hbf
            # Interleave x/h piece loads so the transpose chain (which needs
            # x piece k AND h piece k for group k's matmuls) unblocks early.
            xbf = xin.tile([SUB, spc, I], MM_DT_BIR, tag="xbf", name="xbf")
            hbf = xin.tile([SUB, spc, H], MM_DT_BIR, tag="hbf", name="hbf")
            for p in range(n_pieces):
                sl = slice(p * ppc, (p + 1) * ppc)
                nc.gpsimd.dma_start(xbf[:, sl, :], x_r[n][:, sl, :])  # f32->fp16
                nc.gpsimd.dma_start(hbf[:, sl, :], h_r[n][:, sl, :])
            return xbf, hbf

        def emit_tp(xbf, hbf):
            if xbf is None or (IO_ONLY and "t" not in IO_PARTS):
                return None, None
            # Blocked xbar transpose per input: out[c, r, p] = in[p, r*128+c],
            # i.e. xT[:, r, :] is the transpose of x subtile r.
            xT = xtp.tile([I, spc, SUB], MM_DT_BIR, tag="xT", name="xT")
            hT = xtp.tile([H, spc, SUB], MM_DT_BIR, tag="hT", name="hT")
            for p in range(n_pieces):
                sl = slice(p * ppc, (p + 1) * ppc)
                tp_x_eng.dma_start(xT[:, sl, :], xbf[:, sl, :], transpose=True)
                tp_h_eng.dma_start(hT[:, sl, :], hbf[:, sl, :], transpose=True)
            return xT, hT

        def emit_c_load(n):
            if IO_ONLY and "l" not in IO_PARTS:
                return None
            c_sb = cin.tile([SUB, spc, H], c_dt, tag="c", name="c_sb")
            c_eng.dma_start(c_sb[:], c_r[n])  # gpsimd path casts f32 -> bf16
            return c_sb

        def emit_warmup():
            # PE warmup: input-independent matmuls run at t=0, overlapping the
            # DMA fill, so the HAM clock-gate reaches 2.4 GHz before the real
            # matmuls start (~3.4 us of sustained PE activity required).
            ps_w = psum.tile([SUB, SUBS_PER_GROUP, G], dt.float32, tag="ps")
            for _w in range(WARMUP):
                nc.tensor.matmul(
                    ps_w[:, 0, :], wu[:, 0:SUB], wu[:], start=True, stop=True
                )

        with loop_cm:
         for _rep in range(reps):
          if WARMUP and WARMUP_POS == "pre":
              emit_warmup()
          state = {}
          if PIPE:
              st0 = state[0] = {}
              st0["xh"] = emit_xh_load(0)
              st0["tp"] = emit_tp(*st0["xh"])
              st0["c"] = emit_c_load(0)
          if WARMUP and WARMUP_POS == "mid":
              emit_warmup()
          for n in range(n_chunks):
            if not PIPE:
                st_ = state[n] = {}
                st_["xh"] = emit_xh_load(n)
                st_["tp"] = emit_tp(*st_["xh"])
                st_["c"] = emit_c_load(n)
            xT, hT = state[n]["tp"]
            c_sb = state[n]["c"]
            if IO_ONLY:
                ncw = io_ncw
                nhw = io_nhw
            else:
                ncw = post.tile([SUB, spc, H], post_dt, tag="ncw", name="ncw")
                nhw = post.tile([SUB, spc, H], post_dt, tag="nhw", name="nhw")
            for g in range(GROUPS_PER_CHUNK):
                if PIPE and n + 1 < n_chunks:
                    nx = state.setdefault(n + 1, {})
                    last_g = g == GROUPS_PER_CHUNK - 1
                    if g == 0:
                        nx["xh"] = emit_xh_load(n + 1)
                    elif g == 1 or (last_g and "tp" not in nx):
                        nx["tp"] = emit_tp(*nx["xh"])
                    if (g == 2 or last_g) and "tp" in nx and "c" not in nx:
                        nx["c"] = emit_c_load(n + 1)
                if IO_ONLY:
                    se = STORE_EVERY if STORE_EVERY else GROUPS_PER_CHUNK
                    if "s" in IO_PARTS and (
                        (g + 1) % se == 0 or g == GROUPS_PER_CHUNK - 1
                    ):
                        lo = (g // se) * se * SUBS_PER_GROUP
                        hi = (g + 1) * SUBS_PER_GROUP
                        ssl = slice(lo, hi)
                        st_eng.dma_start(ncv_r[n][:, ssl, :], ncw[:, ssl, :])
                        st_eng.dma_start(nh_r[n][:, ssl, :], nhw[:, ssl, :])
                    continue
                ps = psum.tile([SUB, SUBS_PER_GROUP, G], dt.float32, tag="ps")
                # All x-parts first: they only need xT, so the PE's in-order
                # stream isn't blocked on the hT transpose during the fill.
                if FILL_OPT and BIAS_FIRST and BIAS_MM:
                    # Bias matmuls have no data deps: opening each bank with
                    # them does real PE work during the transpose wait.
                    for i in range(SUBS_PER_GROUP):
                        nc.tensor.matmul(
                            ps[:, i, :], ones_sb, bias_sb,
                            start=True, stop=False,
                        )
                    for i in range(SUBS_PER_GROUP):
                        s = g * SUBS_PER_GROUP + i
                        nc.tensor.matmul(
                            ps[:, i, :], xT[:, s, :], wtx_sb,
                            start=False, stop=False,
                        )
                    for i in range(SUBS_PER_GROUP):
                        s = g * SUBS_PER_GROUP + i
                        nc.tensor.matmul(
                            ps[:, i, :], hT[:, s, :], wth_sb,
                            start=False, stop=True,
                        )
                elif FILL_OPT:
                    for i in range(SUBS_PER_GROUP):
                        s = g * SUBS_PER_GROUP + i
                        nc.tensor.matmul(
                            ps[:, i, :], xT[:, s, :], wtx_sb, start=True, stop=False
                        )
                    for i in range(SUBS_PER_GROUP):
                        s = g * SUBS_PER_GROUP + i
                        nc.tensor.matmul(
                            ps[:, i, :], hT[:, s, :], wth_sb, start=False,
                            stop=not BIAS_MM,
                        )
                    if BIAS_MM:
                        for i in range(SUBS_PER_GROUP):
                            nc.tensor.matmul(
                                ps[:, i, :], ones_sb, bias_sb,
                                start=False, stop=True,
                            )
                else:
                    for i in range(SUBS_PER_GROUP):
                        s = g * SUBS_PER_GROUP + i
                        nc.tensor.matmul(
                            ps[:, i, :], xT[:, s, :], wtx_sb, start=True, stop=False
                        )
                        nc.tensor.matmul(
                            ps[:, i, :], hT[:, s, :], wth_sb, start=False,
                            stop=not BIAS_MM,
                        )
                        if BIAS_MM:
                            nc.tensor.matmul(
                                ps[:, i, :], ones_sb, bias_sb,
                                start=False, stop=True,
                            )

                sig = sigp.tile([SUB, SUBS_PER_GROUP, G], sig_dt, tag="sig")
                nc.scalar.activation(sig[:, :, 0:384], ps[:, :, 0:384], AF.Sigmoid)
                nc.scalar.activation(sig[:, :, 384:512], ps[:, :, 384:512], AF.Tanh)

                gsl = slice(g * SUBS_PER_GROUP, (g + 1) * SUBS_PER_GROUP)
                c_sl = c_sb[:, gsl, :]
                ncw_sl = ncw[:, gsl, :]
                nhw_sl = nhw[:, gsl, :]
                m1 = post.tile([SUB, SUBS_PER_GROUP, H], post_dt, tag="m1")
                nc.vector.tensor_mul(m1[:], c_sl, sig[:, :, 0:128])
                m2 = post.tile([SUB, SUBS_PER_GROUP, H], post_dt, tag="m2")
                nc.vector.tensor_mul(m2[:], sig[:, :, 128:256], sig[:, :, 384:512])
                nc.vector.tensor_add(ncw_sl, m1[:], m2[:])
                th = post.tile([SUB, SUBS_PER_GROUP, H], post_dt, tag="th")
                nc.scalar.activation(th[:], ncw_sl, AF.Tanh)
                nc.vector.tensor_mul(nhw_sl, th[:], sig[:, :, 256:384])

                se = STORE_EVERY if STORE_EVERY else GROUPS_PER_CHUNK
                if (g + 1) % se == 0 or g == GROUPS_PER_CHUNK - 1:
                    lo = (g // se) * se * SUBS_PER_GROUP
                    hi = (g + 1) * SUBS_PER_GROUP
                    ssl = slice(lo, hi)
                    st_eng.dma_start(ncv_r[n][:, ssl, :], ncw[:, ssl, :])
                    st_eng.dma_start(nh_r[n][:, ssl, :], nhw[:, ssl, :])

    nc.compile()
    return nc


def _get_program(rows):
    if rows not in _cache:
        _cache[rows] = _build(rows)
    return _cache[rows]


def _host_prep(W1, b1, W2, b2, Wf, bf, W3, b3):
    # Gate packing along the 512-wide output dim: [s1 | s2 | s3 | fl] so the
    # three sigmoid gates are contiguous for one ScalarE op. wtx|wth packed
    # into one tensor (single weight DMA); bias|ones likewise.
    wtx = np.concatenate(
        [W1[:, :I].T, W2[:, :I].T, W3[:, :I].T, Wf[:, :I].T], axis=1
    ).astype(MM_DT)
    wth = np.concatenate(
        [W1[:, I:].T, W2[:, I:].T, W3[:, I:].T, Wf[:, I:].T], axis=1
    ).astype(MM_DT)
    wtxh = np.concatenate([wtx, wth], axis=1)
    bias = np.concatenate([b1, b2, b3, bf]).reshape(1, G).astype(MM_DT)
    ones = np.ones((1, SUB), MM_DT)
    bo = np.concatenate([bias, ones], axis=1)
    return wtxh, bo


def _make_runner(nc):
    """Cached jitted SPMD executor for `nc` (mirrors bass2jax.run_bass_via_pjrt
    but without output-buffer donation so device-resident inputs can be reused
    across timing calls)."""
    import jax
    import concourse.mybir as mybir
    from jax.experimental.shard_map import shard_map
    from jax.sharding import Mesh, PartitionSpec
    from concourse.bass2jax import (
        _bass_exec_p,
        install_neuronx_cc_hook,
        partition_id_tensor,
    )

    install_neuronx_cc_hook()
    assert nc.dbg_addr is None
    partition_name = nc.partition_id_tensor.name if nc.partition_id_tensor else None

    in_names, out_names, out_avals, zero_outs = [], [], [], []
    for alloc in nc.m.functions[0].allocations:
        if not isinstance(alloc, mybir.MemoryLocationSet):
            continue
        name = alloc.memorylocations[0].name
        if alloc.kind == "ExternalInput":
            if name != partition_name:
                in_names.append(name)
        elif alloc.kind == "ExternalOutput":
            out_names.append(name)
            shape = tuple(alloc.tensor_shape)
            dtype = mybir.dt.np(alloc.dtype)
            out_avals.append(jax.core.ShapedArray(shape, dtype))
            zero_outs.append(np.zeros(shape, dtype))
    n_params = len(in_names)
    all_names = in_names + out_names
    if partition_name is not None:
        all_names = all_names + [partition_name]

    def _body(*args):
        operands = list(args)
        if partition_name is not None:
            operands.append(partition_id_tensor())
        outs = _bass_exec_p.bind(
            *operands,
            out_avals=tuple(out_avals),
            in_names=tuple(all_names),
            out_names=tuple(out_names),
            lowering_input_output_aliases=(),
            sim_require_finite=True,
            sim_require_nnan=True,
            nc=nc,
        )
        return tuple(outs)

    devices = jax.devices()[:N_CORES]
    mesh = Mesh(np.asarray(devices), ("core",))
    n_all = n_params + len(out_names)
    sharded = jax.jit(
        shard_map(
            _body,
            mesh=mesh,
            in_specs=(PartitionSpec("core"),) * n_all,
            out_specs=(PartitionSpec("core"),) * len(out_names),
            check_rep=False,
        ),
        keep_unused=True,
    )
    return sharded, in_names, out_names, zero_outs


def _stage_inputs(in_maps, in_names, zero_outs):
    import jax

    concat_in = [
        np.concatenate([m[name] for m in in_maps], axis=0) for name in in_names
    ]
    concat_zeros = [
        np.zeros((N_CORES * z.shape[0], *z.shape[1:]), z.dtype) for z in zero_outs
    ]
    return [jax.device_put(a) for a in concat_in + concat_zeros]


def bench(
    x, h, c, W1, b1, W2, b2, Wf, bf, W3, b3, loop_lo=2048, loop_hi=6144, n_calls=4,
    reps=1,
):
    """Measure per-invocation HW time via wall-clock differencing between two
    device-side-looped builds (loop_lo vs loop_hi iterations), which cancels
    the per-call dispatch overhead. Returns (kernel_ns, tlo_list, thi_list)."""
    import time as _time

    import jax

    x = np.ascontiguousarray(x, np.float32)
    h = np.ascontiguousarray(h, np.float32)
    c = np.ascontiguousarray(c, np.float32)
    wtxh, bo = _host_prep(W1, b1, W2, b2, Wf, bf, W3, b3)
    rows = x.shape[0] // N_CORES
    in_maps = []
    for k in range(N_CORES):
        sl = slice(k * rows, (k + 1) * rows)
        in_maps.append(dict(x=x[sl], h=h[sl], c=c[sl], wtxh=wtxh, bo=bo))

    results = {}
    for loop_n in (loop_lo, loop_hi):
        nc = _build(rows, reps=reps, loop_n=loop_n)
        sharded, in_names, out_names, zero_outs = _make_runner(nc)
        dev_args = _stage_inputs(in_maps, in_names, zero_outs)
        outs = sharded(*dev_args)  # warmup/compile
        jax.block_until_ready(outs)
        times = []
        for _ in range(n_calls):
            t0 = _time.perf_counter()
            outs = sharded(*dev_args)
            jax.block_until_ready(outs)
            times.append((_time.perf_counter() - t0) * 1e9)
        results[loop_n] = times
    tlo = min(results[loop_lo])
    thi = min(results[loop_hi])
    kernel_ns = (thi - tlo) / (loop_hi - loop_lo)
    return kernel_ns, results[loop_lo], results[loop_hi]


def kernel(x, h, c, W1, b1, W2, b2, Wf, bf, W3, b3):
    from concourse.bass_utils import run_bass_kernel_spmd

    global LAST_EXEC_NS
    x = np.ascontiguousarray(x, np.float32)
    h = np.ascontiguousarray(h, np.float32)
    c = np.ascontiguousarray(c, np.float32)
    wtxh, bo = _host_prep(W1, b1, W2, b2, Wf, bf, W3, b3)

    rows = x.shape[0] // N_CORES
    nc = _get_program(rows)

    in_maps = []
    for k in range(N_CORES):
        sl = slice(k * rows, (k + 1) * rows)
        in_maps.append({"x": x[sl], "h": h[sl], "c": c[sl], "wtxh": wtxh, "bo": bo})

    res = run_bass_kernel_spmd(
        nc, in_maps, core_ids=list(range(N_CORES)), trace=TRACE
    )
    LAST_EXEC_NS = res.exec_time_ns

    new_h = np.concatenate([res.results[k]["new_h"] for k in range(N_CORES)], axis=0)
    new_c = np.concatenate([res.results[k]["new_c"] for k in range(N_CORES)], axis=0)
    return new_h, new_c

